# revision 1
# baseline (speedup 1.0000x reference)
"""Single-head causal attention with softmax over the QUERY axis (dim=1).

out[b,i,d] = sum_j softmax_i(mask(q@kT/8))[i,j] * v[j,d]

Strategy: data-parallel over batch B=8, one batch element per NeuronCore.
Per core:
  - transpose x[b] (PE transposes) -> xT [C=384, S=2048]
  - qT = (Wq/8).T @ xT, kT = Wk.T @ xT   (both [64, 2048], d on partitions)
  - v  = x @ Wv                          ([2048, 64] as 16 [128,64] tiles)
  - for each key tile jt: scoresT[j, i] = kT_jt.T @ qT  (j on partitions,
    i on free axis) => softmax over i is a FREE-AXIS reduction, fused into
    the Exp activation via accum_out.  Causal mask handled additively on
    the diagonal block only (i >= j valid).
  - fold 1/denom_j into v rows: vs[j,:] = v[j,:] / denom[j], then
    out[i,:] += attnT_jt[:, i].T @ vs  accumulated in PSUM across jt.
"""

import numpy as np
import sys

sys.path.insert(0, "/opt/trn_rl_repo")

import concourse.bass as bass
import concourse.mybir as mybir
from concourse.bacc import Bacc
from concourse.tile import TileContext
from concourse.bass_utils import run_bass_kernel_spmd

B, S, C, D = 8, 2048, 384, 64
P = 128
NT = S // P  # 16 query/key tiles
CC = C // P  # 3 contraction chunks
F32 = mybir.dt.float32
F32R = mybir.dt.float32r
AFT = mybir.ActivationFunctionType
AX = mybir.AxisListType

_COMPILED = None
BUFS = {"ps": 6, "attnp": 3, "small": 6, "xsp": 4}


def build_nc():
    nc = Bacc()
    x_b = nc.declare_dram_parameter("x_b", [S, C], F32R, isOutput=False)
    wq = nc.declare_dram_parameter("wq", [C, D], F32R, isOutput=False)  # pre-scaled 1/8
    wk = nc.declare_dram_parameter("wk", [C, D], F32R, isOutput=False)
    wv = nc.declare_dram_parameter("wv", [C, D], F32, isOutput=False)
    ident = nc.declare_dram_parameter("ident", [P, P], F32R, isOutput=False)
    negmask = nc.declare_dram_parameter("negmask", [P, P], F32, isOutput=False)
    out_b = nc.declare_dram_parameter("out_b", [S, D], F32, isOutput=True)

    with TileContext(nc) as tc:
        with (
            tc.tile_pool(name="consts", bufs=1) as consts,
            tc.tile_pool(name="big", bufs=1) as big,
            tc.tile_pool(name="xsp", bufs=BUFS["xsp"]) as xsp,
            tc.tile_pool(name="attnp", bufs=BUFS["attnp"]) as attnp,
            tc.tile_pool(name="small", bufs=BUFS["small"]) as small,
            tc.tile_pool(name="psO", bufs=1, space="PSUM") as psO,
            tc.tile_pool(name="ps", bufs=BUFS["ps"], space="PSUM") as ps,
        ):
            # ---- constants ----
            idt = consts.tile([P, P], F32R)
            nc.sync.dma_start(out=idt, in_=ident[:, :])
            msk = consts.tile([P, P], F32)
            nc.sync.dma_start(out=msk, in_=negmask[:, :])
            wq_t = consts.tile([P, CC * D], F32R)
            wk_t = consts.tile([P, CC * D], F32R)
            wv_t = consts.tile([P, CC * D], F32)
            for wt, wd in ((wq_t, wq), (wk_t, wk), (wv_t, wv)):
                nc.sync.dma_start(
                    out=wt.rearrange("p (c d) -> p c d", c=CC),
                    in_=wd.ap().rearrange("(c p) d -> p c d", p=P),
                )

            # ---- persistent SBUF tensors ----
            xT = big.tile([P, CC * S], F32R)        # [128, 3*2048] xT chunks
            qk = big.tile([64, 2 * S], F32R)        # qT(scaled) | kT
            v_all = big.tile([P, NT * D], F32)     # v tiles [128, 16*64]
            out_sb = big.tile([P, NT * D], F32)    # final out staging

            # ---- phase A: load + transpose x (4 s-tiles per DMA) ----
            for g in range(NT // 4):
                xs = xsp.tile([P, 4 * C], F32R, tag="xs")
                nc.sync.dma_start(
                    out=xs.rearrange("p (t c) -> p t c", t=4),
                    in_=x_b[g * 4 * P:(g + 1) * 4 * P, :].rearrange(
                        "(t p) c -> p t c", p=P),
                )
                for c in range(CC):
                    pt4 = ps.tile([P, 512], F32, tag="ps")
                    for t in range(4):
                        nc.tensor.matmul(
                            pt4[:, t * P:(t + 1) * P].bitcast(F32R),
                            xs[:, t * C + c * P: t * C + (c + 1) * P], idt,
                            is_transpose=True, start=(t == 0), stop=(t == 3),
                        )
                    nc.vector.tensor_copy(
                        xT[:, c * S + g * 4 * P: c * S + (g + 1) * 4 * P], pt4
                    )

            # ---- qT / kT: [64, 2048] = W.T @ xT ----
            for n in range(S // 512):
                pq = ps.tile([64, 512], F32, tag="ps")
                for c in range(CC):
                    nc.tensor.matmul(
                        pq, wq_t[:, c * D:(c + 1) * D],
                        xT[:, c * S + n * 512: c * S + (n + 1) * 512],
                        start=(c == 0), stop=(c == CC - 1),
                    )
                nc.vector.tensor_copy(qk[:, n * 512:(n + 1) * 512], pq)
                pk = ps.tile([64, 512], F32, tag="ps")
                for c in range(CC):
                    nc.tensor.matmul(
                        pk, wk_t[:, c * D:(c + 1) * D],
                        xT[:, c * S + n * 512: c * S + (n + 1) * 512],
                        start=(c == 0), stop=(c == CC - 1),
                    )
                nc.vector.tensor_copy(qk[:, S + n * 512: S + (n + 1) * 512], pk)

            # ---- v tiles [128, 64] = xT_chunk.T @ Wv ----
            for st in range(NT):
                pv = ps.tile([P, D], F32, tag="ps")
                for c in range(CC):
                    nc.tensor.matmul(
                        pv, xT[:, c * S + st * P: c * S + (st + 1) * P].bitcast(F32),
                        wv_t[:, c * D:(c + 1) * D],
                        start=(c == 0), stop=(c == CC - 1),
                    )
                nc.vector.tensor_copy(v_all[:, st * D:(st + 1) * D], pv)

            # ---- phase B: per key-tile softmax + accumulation ----
            # Software-pipelined: scores+exp for jt+1 are emitted BEFORE the
            # softmax tail + attn@v matmuls of jt, so PE works on scores_{jt+1}
            # while ACT/DVE finish the softmax chain of jt.
            outp = psO.tile([P, NT * D], F32)  # [128, 1024] accumulator, 2 banks

            def emit_scores(jt):
                Ni = S - jt * P  # valid queries i >= jt*128
                atile = attnp.tile([P, S], F32, tag="attn", name=f"atile{jt}")
                dens = small.tile([P, 4], F32, tag="dens", name=f"dens{jt}")
                nch = (Ni + 511) // 512
                for ci in range(nch):
                    w = min(512, Ni - ci * 512)
                    i0 = jt * P + ci * 512
                    sc = ps.tile([P, 512], F32, tag="ps", name=f"sc{jt}_{ci}")
                    nc.tensor.matmul(
                        sc[:, :w],
                        qk[:, S + jt * P: S + (jt + 1) * P],
                        qk[:, i0: i0 + w],
                        start=True, stop=True,
                    )
                    if ci == 0:
                        # causal mask on diagonal block: -1e30 where i < j
                        nc.vector.tensor_add(sc[:, :P], sc[:, :P], msk)
                    nc.scalar.activation(
                        atile[:, ci * 512: ci * 512 + w], sc[:, :w], AFT.Exp,
                        accum_out=dens[:, ci: ci + 1],
                    )
                return atile, dens, nch

            pend = emit_scores(0)
            for jt in range(NT):
                atile, dens, nch = pend
                if jt + 1 < NT:
                    pend = emit_scores(jt + 1)
                if nch == 1:
                    den = dens[:, 0:1]  # single chunk: accum_out IS the row sum
                else:
                    den_t = small.tile([P, 1], F32, tag="den")
                    nc.vector.reduce_sum(den_t, dens[:, :nch], axis=AX.X)
                    den = den_t
                rv = small.tile([P, 1], F32, tag="rv")
                nc.vector.reciprocal(rv, den)
                vs = small.tile([P, D], F32, tag="vs")
                nc.vector.tensor_scalar_mul(vs, v_all[:, jt * D:(jt + 1) * D], rv)
                for it in range(jt, NT):
                    # outp is 2 PSUM banks (it 0..7 | 8..15). start=True zeroes
                    # the whole 2KB bank, so only the first matmul touching each
                    # bank starts; the last touching each bank stops.
                    bank_first = jt == 0 and it in (0, 8)
                    bank_last = (jt == 7 and it == 7) or (jt == 15 and it == 15)
                    nc.tensor.matmul(
                        outp[:, it * D:(it + 1) * D],
                        atile[:, (it - jt) * P:(it - jt + 1) * P],  # [128j,128i]
                        vs,
                        start=bank_first, stop=bank_last,
                    )

            nc.vector.tensor_copy(out_sb, outp)
            nc.sync.dma_start(
                out=out_b.ap().rearrange("(t p) d -> p t d", p=P),
                in_=out_sb.rearrange("p (t d) -> p t d", t=NT),
            )
    nc.finalize()
    return nc


def _build_inputs(x, Wq, Wk, Wv):
    x = np.ascontiguousarray(np.asarray(x, dtype=np.float32))
    wq_s = np.ascontiguousarray(np.asarray(Wq, dtype=np.float32) * np.float32(D ** -0.5))
    wk_ = np.ascontiguousarray(np.asarray(Wk, dtype=np.float32))
    wv_ = np.ascontiguousarray(np.asarray(Wv, dtype=np.float32))
    ident = np.eye(P, dtype=np.float32)
    r = np.arange(P)
    negmask = np.where(r[None, :] >= r[:, None], 0.0, -1e30).astype(np.float32)
    return [
        {"x_b": x[b], "wq": wq_s, "wk": wk_, "wv": wv_,
         "ident": ident, "negmask": negmask}
        for b in range(B)
    ]


def kernel(x, Wq, Wk, Wv, _trace=False):
    global _COMPILED
    if _COMPILED is None:
        _COMPILED = build_nc()
    nc = _COMPILED
    in_maps = _build_inputs(x, Wq, Wk, Wv)
    res = run_bass_kernel_spmd(nc, in_maps, core_ids=list(range(B)), trace=_trace)
    out = np.stack([res.results[b]["out_b"] for b in range(B)], axis=0).astype(np.float32)
    if _trace:
        kernel.last_results = res
    return out



# revision 23
# speedup vs baseline: 1.7004x; 1.7004x over previous
"""Single-head causal attention, softmax over the QUERY axis (dim=1).

out[b,i,d] = sum_j exp(s[i,j])/den[j] * v[j,d],  den[j] = sum_i exp(s[i,j])

Data-parallel over B=8, one batch per core. fp16 matmul operands (fp32
PSUM accum), host-side pre-transpose of x, scores pre-scaled by
1024*log2(e)/8 via Wq so the DVE/Pool "schraudolph" exp (int16 bias-add
+ bitcast to fp16, max-0 clamp handles the causal mask's -1e30) is one
tensor_scalar op; ACT runs exact Exp (scale=1/SCL) on the rest. q and k
stay stacked on partitions ([q|k] rows of one SBUF tile); the scores
matmul reads kT at partition offset 64 via tile_position. jt runs
descending; a small "jumpstart" DMA of x cols 1536:2048 lets the jt
15..12 pipeline start before the bulk of x arrives.
"""

import numpy as np
import sys

sys.path.insert(0, "/opt/trn_rl_repo")

import concourse.bass as bass
import concourse.mybir as mybir
from concourse.alu_op_type import AluOpType
from concourse.bacc import Bacc
from concourse.tile import TileContext
from concourse.bass_utils import run_bass_kernel_spmd

B, S, C, D = 8, 2048, 384, 64
P = 128
NT = S // P   # 16 key tiles
CC = C // P   # 3 contraction chunks
F32 = mybir.dt.float32
F16 = mybir.dt.float16
I16 = mybir.dt.int16
AFT = mybir.ActivationFunctionType
AX = mybir.AxisListType

LOG2E = 1.4426950408889634
SCL = 1024.0 * LOG2E            # scores arrive in PSUM pre-scaled by this
SBIAS = 15360.0 - 44.0          # fp16 exponent bias<<10, centered correction

# exp engine per (jt, chunk): "A" = exact Exp on ACT;
# "S" = schraudolph tensor_scalar + reduce, both on DVE (GPSIMD cannot
# touch PSUM on real hw, so only ACT/DVE can read scores)
EXP_ASSIGN = {}
for _jt in (15, 14, 13, 12, 11, 10):
    EXP_ASSIGN[(_jt, 0)] = "A"
for _jt in (9, 8):
    EXP_ASSIGN[(_jt, 0)] = "S"
for _jt in range(0, 8):
    EXP_ASSIGN[(_jt, 0)] = "A"
EXP_ASSIGN[(0, 1)] = "A"
for _jt in range(1, 8):
    EXP_ASSIGN[(_jt, 1)] = "S"

# psum->sbuf copy split: q rows 0:64 -> qsb (DVE), k rows 64:128 -> ksb (ACT)

_COMPILED = None


def _eng(nc, code):
    return {"A": nc.scalar, "D": nc.vector, "P": nc.gpsimd}[code]


def _copy(nc, code, dst, src):
    if code == "A":
        nc.scalar.copy(dst, src)
    else:
        _eng(nc, code).tensor_copy(dst, src)


def build_nc():
    nc = Bacc()
    x_cs = nc.declare_dram_parameter("x_cs", [C, S], F16, isOutput=False)
    # packed: per c-chunk 192 cols [Wqk|Wv], then ident[128], negmask[128]
    wqkv = nc.declare_dram_parameter("wqkv", [P, CC * 192 + 2 * P], F16,
                                     isOutput=False)
    out_b = nc.declare_dram_parameter("out_b", [S, D], F16, isOutput=True)

    with TileContext(nc) as tc:
        with (
            tc.tile_pool(name="consts", bufs=1) as consts,
            tc.tile_pool(name="big", bufs=1) as big,
            tc.tile_pool(name="attnp", bufs=4) as attnp,
            tc.tile_pool(name="small", bufs=4) as small,
            tc.tile_pool(name="osbp", bufs=2) as osbp,
            tc.tile_pool(name="ps", bufs=1, space="PSUM") as ps,
        ):
            # preload ACT's Exp table off the critical path
            warm = consts.tile([P, 1], F32)
            nc.vector.memset(warm, 0.0)
            nc.scalar.activation(warm, warm, AFT.Exp)

            wt = consts.tile([P, CC * 192 + 2 * P], F16)
            nc.sync.dma_start(out=wt, in_=wqkv[:, :])
            idm = wt[:, CC * 192: CC * 192 + P]          # identity fp16
            msk = wt[:, CC * 192 + P: CC * 192 + 2 * P]  # -57344 lower tri
            xT = big.tile([P, CC * S], F16)
            xT3 = xT.rearrange("p (c f) -> p c f", c=CC)
            xc3 = x_cs.ap().rearrange("(c p) f -> p c f", p=P)
            # col ranges, most-urgent first: jumpstart (n3), n2, then h0
            nc.sync.dma_start(out=xT3[:, :, 1536:2048], in_=xc3[:, :, 1536:2048])
            nc.sync.dma_start(out=xT3[:, :, 1024:1536], in_=xc3[:, :, 1024:1536])
            nc.sync.dma_start(out=xT3[:, :, 0:1024], in_=xc3[:, :, 0:1024])

            qsb = big.tile([64, S], F16)
            ksb = big.tile([64, S], F16)
            vsb = big.tile([P, NT * D], F32)

            def qk_copies(pt, c0, c1):
                nc.vector.tensor_copy(qsb[:, c0:c1], pt[0:64, 0:c1 - c0])
                nc.scalar.copy(ksb[:, c0:c1], pt[64:128, 0:c1 - c0])

            def sct(name):
                return ps.tile([P, 1024], F32, tag="sc", bufs=3, name=name)

            def qk_mms(pt, n):  # n-chunk of 512 cols into pt[:, 0:512]
                for c in range(CC):
                    nc.tensor.matmul(
                        pt[:, 0:512], wt[:, c * 192: c * 192 + P],
                        xT[:, c * S + n * 512:c * S + (n + 1) * 512],
                        start=(c == 0), stop=(c == CC - 1),
                    )

            def v_mms(pt, st0, n_st, base_st, first, last):
                # st0..st0+n_st-1 -> pt cols (st-base_st)*64
                for c in range(CC):
                    for st in range(st0, st0 + n_st):
                        o = (st - base_st) * 64
                        nc.tensor.matmul(
                            pt[:, o:o + 64],
                            xT[:, c * S + st * P: c * S + (st + 1) * P],
                            wt[:, c * 192 + P:(c + 1) * 192],
                            start=(c == 0 and st == st0 and first),
                            stop=(c == CC - 1 and st == st0 + n_st - 1
                                  and last),
                        )

            outp = ps.tile([P, NT * D], F32, tag="out", bufs=1)

            def emit_block(jt):
                Ni = S - jt * P
                nch = (Ni + 1023) // 1024
                atile = attnp.tile([P, S], F16, tag="attn", name=f"at{jt}")
                dens = small.tile([P, 2], F32, tag="dens", name=f"dens{jt}")
                for ci in range(nch):
                    w = min(1024, Ni - ci * 1024)
                    i0 = jt * P + ci * 1024
                    scp = sct(f"sc{jt}_{ci}")
                    for hh in range(0, w, 512):
                        hw_ = min(512, w - hh)
                        diag = ci == 0 and hh == 0
                        nc.tensor.matmul(
                            scp[:, hh:hh + hw_],
                            ksb[:, jt * P:(jt + 1) * P],
                            qsb[:, i0 + hh: i0 + hh + hw_],
                            start=True, stop=not diag,
                        )
                        if diag:
                            # causal mask on PE: += I.T @ negmask over the
                            # diagonal 128 cols (GPSIMD can't touch PSUM)
                            nc.tensor.matmul(
                                scp[:, 0:P], idm, msk,
                                start=False, stop=True,
                            )
                    eng = EXP_ASSIGN.get((jt, ci), "A")
                    oap = atile[:, ci * 1024: ci * 1024 + w]
                    if eng == "A":
                        nc.scalar.activation(
                            oap, scp[:, :w], AFT.Exp, scale=1.0 / SCL,
                            accum_out=dens[:, ci:ci + 1],
                        )
                    else:
                        nc.vector.tensor_scalar(
                            oap.bitcast(I16), scp[:, :w],
                            SBIAS, 0.0, AluOpType.add, AluOpType.max,
                        )
                        nc.vector.reduce_sum(dens[:, ci:ci + 1], oap, axis=AX.X)
                if nch > 1:
                    den = small.tile([P, 1], F32, tag="den", name=f"den{jt}")
                    nc.vector.reduce_sum(den, dens[:, :nch], axis=AX.X)
                else:
                    den = dens[:, 0:1]
                rv = small.tile([P, 1], F32, tag="rv", name=f"rv{jt}")
                nc.vector.reciprocal(rv, den)
                vs = small.tile([P, D], F16, tag="vs", name=f"vs{jt}")
                nc.gpsimd.tensor_scalar_mul(
                    vs, vsb[:, jt * D:(jt + 1) * D], rv)
                return atile, vs

            def emit_attnv(jt, blk):
                atile, vs = blk
                for it in range(jt, NT):
                    nc.tensor.matmul(
                        outp[:, it * D:(it + 1) * D],
                        atile[:, (it - jt) * P:(it - jt + 1) * P],
                        vs,
                        start=(jt == 15 and it == 15) or (jt == 7 and it == 7),
                        stop=(jt == 0 and it in (7, 15)),
                    )

            blocks = {}

            # ---- phase 1: jts 15..8 (needs only x cols >= 1024) ----
            pm = sct("mini")              # n3 jumpstart
            qk_mms(pm, 3)
            nc.vector.tensor_copy(qsb[:, 1920:2048], pm[0:64, 384:512])
            nc.scalar.copy(ksb[:, 1920:2048], pm[64:128, 384:512])
            nc.vector.tensor_copy(qsb[:, 1536:1920], pm[0:64, 0:384])
            nc.scalar.copy(ksb[:, 1536:1920], pm[64:128, 0:384])
            # v st 12..15 -> outp bank0 cols 256:512 (av's jt7-start re-zeroes
            # bank0 much later; the copies below overlap cols 448:512 so the
            # WAR edge orders that zeroing after them)
            # v st 12..15 -> outp bank0 cols 256:512 (complete group; av's
            # jt7-start re-zeroes bank0 much later — the copy's read of cols
            # 256:512 overlaps that start-write at 448:512, ordering them)
            v_mms(outp, 12, 4, 8, first=True, last=True)
            _copy(nc, "D", vsb[:, 768:1024], outp[:, 256:512])
            blocks[15] = emit_block(15)
            blocks[14] = emit_block(14)
            blocks[13] = emit_block(13)
            blocks[12] = emit_block(12)
            pq2 = sct("pq2")
            qk_mms(pq2, 2)
            qk_copies(pq2, 1024, 1536)
            pvm = sct("pvm")
            v_mms(pvm, 8, 4, 8, first=True, last=True)     # st 8..11 -> 0:256
            _copy(nc, "D", vsb[:, 512:768], pvm[:, 0:256])
            emit_attnv(15, blocks.pop(15))
            for jt in (11, 10, 9, 8):
                blocks[jt] = emit_block(jt)
                emit_attnv(jt + 3, blocks.pop(jt + 3))

            # ---- phase 2: jts 7..0 (needs all of x) ----
            pq0 = sct("pq0")
            qk_mms(pq0, 0)
            pq1 = sct("pq1")
            qk_mms(pq1, 1)
            qk_copies(pq0, 0, 512)
            qk_copies(pq1, 512, 1024)
            pv0 = sct("pv0")
            v_mms(pv0, 0, 8, 0, first=True, last=True)
            _copy(nc, "D", vsb[:, 0:512], pv0[:, 0:512])
            for jt in (7, 6, 5, 4, 3, 2, 1, 0):
                blocks[jt] = emit_block(jt)
                emit_attnv(jt + 3, blocks.pop(jt + 3))
            for jt in (2, 1, 0):
                emit_attnv(jt, blocks.pop(jt))

            # out: bank0 = it 0..7 (cols 0:512), bank1 = it 8..15
            for hb in range(2):
                osb = osbp.tile([P, 512], F16, tag="osb", name=f"osb{hb}")
                _copy(nc, "A" if hb == 0 else "D", osb,
                      outp[:, hb * 512:(hb + 1) * 512])
                nc.sync.dma_start(
                    out=out_b[hb * 1024:(hb + 1) * 1024, :].rearrange(
                        "(t p) d -> p t d", p=P),
                    in_=osb.rearrange("p (t d) -> p t d", t=8),
                )
    nc.finalize()
    return nc


def _build_inputs(x, Wq, Wk, Wv):
    x = np.asarray(x, dtype=np.float32)
    wq = np.asarray(Wq, dtype=np.float32) * np.float32(SCL * D ** -0.5)
    wk = np.asarray(Wk, dtype=np.float32)
    wv = np.asarray(Wv, dtype=np.float32)
    wqkv = np.zeros((P, CC * 192 + 2 * P), dtype=np.float16)
    for c in range(CC):
        rows = slice(c * P, (c + 1) * P)
        wqkv[:, c * 192: c * 192 + 64] = wq[rows].astype(np.float16)
        wqkv[:, c * 192 + 64: c * 192 + 128] = wk[rows].astype(np.float16)
        wqkv[:, c * 192 + 128:(c + 1) * 192] = wv[rows].astype(np.float16)
    wqkv[:, CC * 192: CC * 192 + P] = np.eye(P, dtype=np.float16)
    r = np.arange(P)
    # [k, i] = -57344 where i < k (query i before key k -> masked)
    wqkv[:, CC * 192 + P:] = np.where(
        r[None, :] >= r[:, None], 0.0, -57344.0).astype(np.float16)
    return [
        {"x_cs": np.ascontiguousarray(x[b].T).astype(np.float16),
         "wqkv": wqkv}
        for b in range(B)
    ]


def kernel(x, Wq, Wk, Wv, _trace=False):
    global _COMPILED
    if _COMPILED is None:
        _COMPILED = build_nc()
    nc = _COMPILED
    in_maps = _build_inputs(x, Wq, Wk, Wv)
    res = run_bass_kernel_spmd(nc, in_maps, core_ids=list(range(B)),
                               trace=_trace)
    out = np.stack([res.results[b]["out_b"] for b in range(B)],
                   axis=0).astype(np.float32)
    if _trace:
        kernel.last_results = res
    return out
